# revision 25
# baseline (speedup 1.0000x reference)
"""Trainium2 Bass kernel for LocalFeatureSamplerV10 (retrieval_knn), v4.

Full-input contract: kernel(**inputs) takes the complete unsharded numpy
inputs and returns the full [32, 512] output. Internally shards the batch
dim over 8 NeuronCores (4 batches/core), replicating the MLP weights.

v4 changes vs v3 (72.4us measured):
  * Distance phase: pairs processed as 4 couples (2 pairs batched per op,
    FD=768); subs and reduces split across vector/gpsimd; squares on
    scalar. Vector busy in stage A drops ~2.7x.
  * All constants + weights packed into 3 large DMAs on the sync queue
    (was 13 small ones, ~7.5us of gpsimd queue issue time freed).
  * MLP layer 1s col-tiled: t=0 at col_grp 0, t=1 at col_grp 1 run
    concurrently in the PE array (M=4 each); bias rank-1 matmuls open
    each accumulation group.
  * MLP layer 2s flipped: W2^T chunks [128h,128o] are the stationary
    operand, hT [128,4] the moving one -> output lands feature-major
    [128o, b], so no PE transposes of the layer-2 output, no activation
    copies; bias is fused into the PSUM->SBUF scalar copies (per-
    partition bias = b2 transposed). Final output written transposed
    [128, 4og, 4b] and unscrambled on host.
  * fw layer 1 N-split into two col_grp chains (halves of the 512 output
    cols), so consecutive matmuls never share a column group.
  * PSUM->SBUF copies (tv/cand/wB) moved to the scalar engine; pool
    reduces split gpsimd(chunk0)/vector(chunk1), dropping the v3
    scheduler-ordering hack.

Per-core algorithm (4 batches x 2 queries = 8 "pairs", pair = t*4 + b):
  1. s = -||p - q||^2 laid out [128 part, 128] per pair (point n = p*128+j).
  2. Top-32 per pair: per-partition top-8 (max8) -> PE-transpose candidates
     -> per-row top-32 (max + match_replace rounds) -> PE-transpose flatten
     -> global top-32 with the 8 pairs stacked on partitions (bit-exact).
  3. Indices via max_index against the original s rows + p*128, cross-
     partition min via PE transpose + reduce_min, clamped, + batch offset;
     FIND outputs land in permuted columns so the result is already in
     dma_gather's wrapped [16, n/16] table order.
  4. Two dma_gather(transpose=True) of 128 rows each from the bf16 feature
     stacks; reduce_max over K -> X [128ch, 8chhi, b, t] bf16.
  5. MLPs as bf16 PE matmuls (details above); fp32 output.
"""

import numpy as np
import ml_dtypes

import concourse.bass as bass
from concourse import bacc
import concourse.mybir as mybir
import concourse.tile as tile

B, N, C, K, OUT = 32, 16384, 1024, 32, 512
H = 512
NCORES = 8
BPC = B // NCORES          # batches per core
P = 128
NP = N // P                # 128 points per partition
NPAIR = 2 * BPC            # 8 (pair = t*BPC + b; 0-3 joint, 4-7 drag)
F32 = mybir.dt.float32
BF16 = mybir.dt.bfloat16
U32 = mybir.dt.uint32
I16 = mybir.dt.int16
NEG = -3.0e38

AX = mybir.AxisListType
OP = mybir.AluOpType
ACTF = mybir.ActivationFunctionType

BF = ml_dtypes.bfloat16

# ---- packed constant-f32 column offsets --------------------------------
CF_IDENT = 0
CF_PBASE = 128
CF_BOFFS = 129
CF_MASK8 = 130
CF_LREP = 138
CF_EREP = 266            # [8 rows, NPAIR*P=1024]
CF_MASKR = 1290          # [64 rows, 8]
CF_SELQ = 1298           # [64 rows, 8]
CF_B2T0 = 1306           # [128, 4]
CF_B2T1 = 1310
CF_FB2T = 1314
CF_TOTAL = 1318

# ---- packed constant-bf16 column offsets -------------------------------
CB_IDENT = 0
CB_ONES = 128            # 8 real ones (row 0)
CB_B10 = 136             # row 0 only, 512 cols
CB_B11 = 648
CB_FB1 = 1160
CB_TOTAL = 1672

# ---- packed weight-bf16 column offsets ---------------------------------
W_W10 = 0                # [128, 8, 512]
W_W11 = 4096
W_W2T0 = 8192            # [128, 4, 4, 128]
W_W2T1 = 10240
W_FW1 = 12288            # [128, 8, 512]
W_FW2T = 16384           # [128, 4, 4, 128]
W_TOTAL = 18432


def build_nc():
    nc = bacc.Bacc(trn_type="TRN2")

    qpts0 = nc.dram_tensor("qpts0", [P, NPAIR * 3 + 2 * NP * 3], F32,
                           kind="ExternalInput")
    qpts1 = nc.dram_tensor("qpts1", [P, 2 * NP * 3], F32, kind="ExternalInput")
    cf = nc.dram_tensor("cf", [P, CF_TOTAL], F32, kind="ExternalInput")
    cb = nc.dram_tensor("cb", [P, CB_TOTAL], BF16, kind="ExternalInput")
    wts = nc.dram_tensor("wts", [P, W_TOTAL], BF16, kind="ExternalInput")
    feats = [nc.dram_tensor(f"feats{h}", [2 * N, C], BF16, kind="ExternalInput")
             for h in range(2)]
    out = nc.dram_tensor("out", [P, 4 * BPC], F32, kind="ExternalOutput")

    with tile.TileContext(nc) as tc:
        _body(tc, nc, qpts0, qpts1, cf, cb, wts, feats, out)
    nc.compile()
    return nc


def _body(tc, nc, qpts0, qpts1, cf, cb, wts, feats, out):
    from contextlib import ExitStack
    with ExitStack() as ctx:
        cpool = ctx.enter_context(tc.tile_pool(name="const", bufs=1))
        wpool = ctx.enter_context(tc.tile_pool(name="weights", bufs=1))
        state = ctx.enter_context(tc.tile_pool(name="state", bufs=1))
        work = ctx.enter_context(tc.tile_pool(name="work", bufs=2))
        psA = ctx.enter_context(tc.tile_pool(name="psA", bufs=1, space="PSUM"))
        psB = ctx.enter_context(tc.tile_pool(name="psB", bufs=1, space="PSUM"))
        psT = ctx.enter_context(tc.tile_pool(name="psT", bufs=1, space="PSUM"))
        psO = ctx.enter_context(tc.tile_pool(name="psO", bufs=4, space="PSUM"))

        # ---- input DMAs, all on the sync queue ---------------------------
        # DMA1: queries + points batches 0,1; DMA2: points batches 2,3
        qp0 = state.tile([P, NPAIR * 3 + 2 * NP * 3], F32, tag="qp0")
        nc.sync.dma_start(out=qp0[:, :], in_=qpts0[:, :])
        qb_s = qp0[:, :NPAIR * 3].rearrange("p (i c) -> p i c", c=3)
        qp1 = state.tile([P, 2 * NP * 3], F32, tag="qp1")
        nc.sync.dma_start(out=qp1[:, :], in_=qpts1[:, :])

        def ptile(b):  # [P, NP, 3] view of batch b's points
            if b < 2:
                sl = qp0[:, NPAIR * 3 + b * NP * 3:NPAIR * 3 + (b + 1) * NP * 3]
            else:
                sl = qp1[:, (b - 2) * NP * 3:(b - 1) * NP * 3]
            return sl.rearrange("p (j c) -> p j c", c=3)

        # DMA3/4: packed constants
        cf_s = cpool.tile([P, CF_TOTAL], F32, tag="cf_s")
        nc.sync.dma_start(out=cf_s[:, :], in_=cf[:, :])
        cb_s = cpool.tile([P, CB_TOTAL], BF16, tag="cb_s")
        nc.sync.dma_start(out=cb_s[:, :], in_=cb[:, :])
        # DMA5: packed weights
        w_s = wpool.tile([P, W_TOTAL], BF16, tag="w_s")
        nc.sync.dma_start(out=w_s[:, :], in_=wts[:, :])

        ident = cf_s[:, CF_IDENT:CF_IDENT + P]
        pbase_s = cf_s[:, CF_PBASE:CF_PBASE + 1]
        boffs_s = cf_s[:, CF_BOFFS:CF_BOFFS + 1]
        mask8_s = cf_s[:, CF_MASK8:CF_MASK8 + 8]
        lrep_s = cf_s[:, CF_LREP:CF_LREP + P]
        erep_s = cf_s[:NPAIR, CF_EREP:CF_EREP + NPAIR * P]
        maskr_s = cf_s[:NPAIR * 8, CF_MASKR:CF_MASKR + 8]
        selq_s = cf_s[:NPAIR * 8, CF_SELQ:CF_SELQ + NPAIR]
        b2T = {0: cf_s[:, CF_B2T0:CF_B2T0 + 4], 1: cf_s[:, CF_B2T1:CF_B2T1 + 4]}
        fb2T = cf_s[:, CF_FB2T:CF_FB2T + 4]

        identb_s = cb_s[:, CB_IDENT:CB_IDENT + P]
        onesb4 = cb_s[:1, CB_ONES:CB_ONES + BPC]
        b1 = {0: cb_s[:1, CB_B10:CB_B10 + H], 1: cb_s[:1, CB_B11:CB_B11 + H]}
        fb1 = cb_s[:1, CB_FB1:CB_FB1 + H]

        w1 = {0: w_s[:, W_W10:W_W10 + 8 * H].rearrange("p (ch o) -> p ch o", ch=8),
              1: w_s[:, W_W11:W_W11 + 8 * H].rearrange("p (ch o) -> p ch o", ch=8)}
        w2T = {0: w_s[:, W_W2T0:W_W2T0 + 2048].rearrange(
                   "p (ic og o) -> p ic og o", ic=4, og=4),
               1: w_s[:, W_W2T1:W_W2T1 + 2048].rearrange(
                   "p (ic og o) -> p ic og o", ic=4, og=4)}
        fw1 = w_s[:, W_FW1:W_FW1 + 8 * H].rearrange("p (ch o) -> p ch o", ch=8)
        fw2T = w_s[:, W_FW2T:W_FW2T + 2048].rearrange(
            "p (ic og o) -> p ic og o", ic=4, og=4)

        # ---- PE warm-up scratch (shares psT's bank) ----------------------
        dps = psT.tile([NPAIR, P], F32, tag="tr", name="dps")

        def warm_f32(anchor_ap):
            kk = anchor_ap.shape[0]
            nc.tensor.matmul(out=dps[:anchor_ap.shape[1], :P], lhsT=anchor_ap,
                             rhs=ident[:kk, :], start=True, stop=True)

        def warm_bf16(anchor_ap):
            nc.tensor.matmul(out=dps[:anchor_ap.shape[1], :], lhsT=anchor_ap,
                             rhs=w1[0][:, 0, :P], start=True, stop=True)

        # ---- stage A: s = -d2 per couple (2 pairs), stage B: top-8 -------
        # couples: Cq = pairs (2q, 2q+1) i.e. (t, b01) with t=q//2? No:
        # pair = t*4+b. Couple layout below: (t, bpair) with bpair in {01, 23}.
        s_all = state.tile([P, NPAIR, NP], F32, tag="s_all")
        v8f = state.tile([P, NPAIR * 8], F32, tag="v8f")

        # couple index list: (t, b0) -> pairs (t*4+b0, t*4+b0+1)
        couples = [(0, 0), (1, 0), (1, 2), (0, 2)]  # emission order
        sub_eng = [nc.vector, nc.gpsimd, nc.vector, nc.gpsimd]
        red_eng = [nc.vector, nc.vector, nc.vector, nc.vector]

        for cidx, (t, b0) in enumerate(couples):
            i0 = t * BPC + b0
            diff = work.tile([P, 2, NP, 3], F32, tag="diff", name=f"diff{cidx}")
            # both batches' points, contiguous; queries broadcast per pair
            for k in range(2):
                sub_eng[cidx].tensor_sub(
                    out=diff[:, k, :, :], in0=ptile(b0 + k),
                    in1=qb_s[:, i0 + k:i0 + k + 1, :].to_broadcast([P, NP, 3]))
            sq = work.tile([P, 2, NP, 3], F32, tag="sq", name=f"sq{cidx}")
            nc.scalar.square(out=sq[:, :, :, :], in_=diff[:, :, :, :])
            red_eng[cidx].tensor_reduce(
                out=s_all[:, i0:i0 + 2, :], in_=sq[:, :, :, :],
                axis=AX.X, op=OP.add, negate=True)
            for k in range(2):
                i = i0 + k
                nc.vector.max(out=v8f[:, i * 8:(i + 1) * 8], in_=s_all[:, i, :])

        # ---- transpose candidates: [128, 64] -> [64, 128] ----------------
        tvp = psA.tile([NPAIR * 8, P], F32, tag="t64", name="tvp")
        nc.tensor.transpose(out=tvp[:, :], in_=v8f[:, :], identity=ident[:, :])
        tv = state.tile([NPAIR * 8, P], F32, tag="tv")
        nc.vector.tensor_copy(tv[:, :], tvp[:, :])

        # ---- stage C: per-row top-32 of candidates -----------------------
        cv = state.tile([NPAIR * 8, 32], F32, tag="cv")
        for r in range(4):
            sl = cv[:, r * 8:(r + 1) * 8]
            nc.vector.max(out=sl, in_=tv[:, :])
            if r < 3:
                nc.vector.match_replace(out=tv[:, :], in_to_replace=sl,
                                        in_values=tv[:, :], imm_value=NEG)

        # ---- flatten [64,32] -> [8,104] via masked matmul ----------------
        # rank-r of a partition can hold at most floor(32/(r+1)) winners, so
        # keep 32/16/16/8.. candidates per rank row (rounded to 8s) = 104.
        NCAND = 104
        cvmask = state.tile([NPAIR * 8, NCAND], F32, tag="cvmask")
        nc.vector.tensor_tensor(
            out=cvmask[:, 0:32], in0=cv[:, 0:32],
            in1=maskr_s[:, 0:1].to_broadcast([NPAIR * 8, 32]), op=OP.mult)
        nc.vector.tensor_tensor(
            out=cvmask[:, 32:64].rearrange("k (a c) -> k a c", a=2),
            in0=cv[:, 0:16].rearrange("k (a c) -> k a c", a=1).to_broadcast(
                [NPAIR * 8, 2, 16]),
            in1=maskr_s[:, 1:3].rearrange("k (r u) -> k r u", u=1).to_broadcast(
                [NPAIR * 8, 2, 16]),
            op=OP.mult)
        nc.vector.tensor_tensor(
            out=cvmask[:, 64:104].rearrange("k (a c) -> k a c", a=5),
            in0=cv[:, 0:8].rearrange("k (a c) -> k a c", a=1).to_broadcast(
                [NPAIR * 8, 5, 8]),
            in1=maskr_s[:, 3:8].rearrange("k (r u) -> k r u", u=1).to_broadcast(
                [NPAIR * 8, 5, 8]),
            op=OP.mult)
        candp = psA.tile([NPAIR, NCAND], F32, tag="t64", name="candp")
        nc.tensor.matmul(out=candp[:, :], lhsT=selq_s[:, :],
                         rhs=cvmask[:, :], start=True, stop=True)
        cand = state.tile([NPAIR, NCAND], F32, tag="cand")
        nc.vector.tensor_copy(cand[:, :], candp[:, :])

        # ---- stage D: global top-32 --------------------------------------
        wv = state.tile([NPAIR, 32], F32, tag="wv")
        for r in range(4):
            sl = wv[:, r * 8:(r + 1) * 8]
            nc.vector.max(out=sl, in_=cand[:, :])
            if r < 3:
                nc.vector.match_replace(out=cand[:, :], in_to_replace=sl,
                                        in_values=cand[:, :], imm_value=NEG)

        # ---- broadcast winners via per-pair selector matmuls -------------
        wBs = {}
        for q in [0, 1, 4, 5, 2, 3, 6, 7]:
            wbp = psA.tile([P, 32], F32, tag="t64", name=f"wbp{q}")
            nc.tensor.matmul(out=wbp[:, :],
                             lhsT=erep_s[:, q * P:(q + 1) * P],
                             rhs=wv[:, :], start=True, stop=True)
            wB = state.tile([P, 32], F32, tag=f"wB{q}", name=f"wB{q}")
            nc.scalar.copy(wB[:, :], wbp[:, :])
            wBs[q] = wB

        # ---- per 2-batch chunk: index recovery + gather + maxpool --------
        ju = state.tile([P, 2 * P], U32, tag="ju")
        jf = state.tile([P, 2 * P], F32, tag="jf")
        gfin = state.tile([P, 2], F32, tag="gfin")
        gcl = state.tile([P, 2], F32, tag="gcl")
        Xhs = [state.tile([P, 4, BPC, 2], BF16, tag=f"Xh{g}", name=f"Xh{g}")
               for g in range(2)]
        gmasks = []
        xgs = []
        idx_insts = []
        for hh in range(2):
            for t in range(2):
                for b2 in range(2):
                    i = t * BPC + 2 * hh + b2
                    for g in range(4):
                        col = hh * 128 + b2 * 64 + t * 32 + g * 8
                        nc.vector.max_index(out=ju[:, col:col + 8],
                                            in_max=wBs[i][:, g * 8:(g + 1) * 8],
                                            in_values=s_all[:, i, :])
            jfh = jf[:, hh * P:(hh + 1) * P]
            # cast u32->f32 and add p*128 in one DVE op
            nc.vector.scalar_tensor_tensor(
                out=jfh, in0=ju[:, hh * P:(hh + 1) * P], scalar=1.0,
                in1=pbase_s[:, :].to_broadcast([P, P]),
                op0=OP.mult, op1=OP.add)
            tp = psA.tile([P, P], F32, tag="t64", name=f"tp{hh}")
            nc.tensor.transpose(out=tp[:, :], in_=jfh, identity=ident[:, :])
            nc.vector.tensor_reduce(out=gfin[:, hh:hh + 1], in_=tp[:, :],
                                    axis=AX.X, op=OP.min)
            # clamp NOT_FOUND (huge) to N-1 and add per-slot batch offset
            nc.vector.scalar_tensor_tensor(
                out=gcl[:, hh:hh + 1], in0=gfin[:, hh:hh + 1],
                scalar=float(N - 1), in1=boffs_s[:, :],
                op0=OP.min, op1=OP.add)
            # wrapped+replicated idx table in one masked matmul
            gmask = state.tile([P, 8], F32, tag=f"gmask{hh}", name=f"gmask{hh}")
            gmasks.append(gmask)
            nc.gpsimd.tensor_tensor(
                out=gmask[:, :], in0=gcl[:, hh:hh + 1].to_broadcast([P, 8]),
                in1=mask8_s[:, :], op=OP.mult)
            Tp = psA.tile([P, 8], F32, tag="t64", name=f"Tp{hh}")
            nc.tensor.matmul(out=Tp[:, :], lhsT=lrep_s[:, :], rhs=gmask[:, :],
                             start=True, stop=True)
            idx16 = state.tile([P, 8], I16, tag=f"idx16_{hh}", name=f"idx16_{hh}")
            idx_insts.append(nc.vector.tensor_copy(idx16[:, :], Tp[:, :]))
            xg = state.tile([P, 8, P], BF16, tag=f"xg{hh}", name=f"xg{hh}")
            nc.gpsimd.dma_gather(
                xg[:, :, :], feats[hh][:, :], idx16[:, :],
                num_idxs=P, num_idxs_reg=P, elem_size=C, transpose=True)
            xgs.append(xg)

        # maxpool over K (vector-only); g-major so L1 ch0-3 start early
        from bass_rust import InstructionNameOrderedSet
        for g in range(2):
            for hh in range(2):
                red = nc.vector.tensor_reduce(
                    out=Xhs[g][:, :, 2 * hh:2 * hh + 2, :],
                    in_=xgs[hh][:, g * 4:(g + 1) * 4, :].rearrange(
                        "p c8 (b2 t w) -> p c8 b2 t w", t=2, w=32),
                    axis=AX.X, op=OP.max)
                if hh == 0:
                    # ordering-only edge: keep chunk 0's reduces out of the
                    # vector stream until chunk 1's tail has issued
                    dep = InstructionNameOrderedSet()
                    dep.add(idx_insts[1].ins.name)
                    red.ins.add_nosync_dependencies_from(dep)

        # PE warm-up across the gather wait
        warm_f32(gmasks[1][:, :])
        for _ in range(14):
            warm_bf16(identb_s[:, :8])

        # ---- MLPs (bf16) -------------------------------------------------
        # t-layer1: col-tiled, t=0 -> rows 0:4 (col_grp 0), t=1 -> rows
        # 32:36 (col_grp 1), separate PSUM banks, concurrent in the array.
        ps1 = {0: psB.tile([36, H], F32, tag="ps1a", name="ps1a"),
               1: psB.tile([36, H], F32, tag="ps1b", name="ps1b")}
        for t in range(2):
            nc.tensor.matmul(out=ps1[t][32 * t:32 * t + BPC, :], lhsT=onesb4,
                             rhs=b1[t], start=True, stop=False)
        for ch in range(8):
            for t in range(2):
                nc.tensor.matmul(
                    out=ps1[t][32 * t:32 * t + BPC, :],
                    lhsT=Xhs[ch // 4][:, ch % 4, :, t],
                    rhs=w1[t][:, ch, :], start=False, stop=(ch == 7))
        hs = {}
        for t in range(2):
            ht = state.tile([BPC, H], BF16, tag=f"h{t}", name=f"h{t}")
            if t == 0:
                nc.vector.tensor_scalar_max(ht[:, :],
                                            ps1[t][0:BPC, :], 0.0)
            else:
                nc.scalar.activation(out=ht[:, :],
                                     in_=ps1[t][32:32 + BPC, :],
                                     func=ACTF.Relu)
            hs[t] = ht

        # transpose h pair [4, 512] -> hT [128, 4ic, (t,b)=8]
        hT = state.tile([P, 4, NPAIR], BF16, tag="hT")
        for ic2 in range(2):
            hTp = psT.tile([P, 2, NPAIR], BF16, tag="tr", name=f"hTp{ic2}")
            for j in range(2):
                ic = ic2 * 2 + j
                for t in range(2):
                    nc.tensor.transpose(
                        out=hTp[:, j, t * BPC:(t + 1) * BPC],
                        in_=hs[t][:, ic * P:(ic + 1) * P],
                        identity=identb_s[:BPC, :BPC])
            nc.vector.tensor_copy(hT[:, ic2 * 2:ic2 * 2 + 2, :], hTp[:, :, :])

        # t-layer2 flipped: stationary W2^T chunks, out feature-major.
        # 8 groups (t, og), 4 accumulating matmuls each, interleaved so
        # consecutive matmuls hit different PSUM tiles.
        # bias-fused copies target cT [128, (t,og)=8, b]
        cT = state.tile([P, 2 * 4, BPC], BF16, tag="cT")
        for t in range(2):
            o_ps = [psO.tile([P, BPC], F32, tag="ops", name=f"ops{t}{og}")
                    for og in range(4)]
            for ic in range(4):
                for og in range(4):
                    nc.tensor.matmul(
                        out=o_ps[og][:, :],
                        lhsT=w2T[t][:, ic, og, :],
                        rhs=hT[:, ic, t * BPC:(t + 1) * BPC],
                        start=(ic == 0), stop=(ic == 3))
            for og in range(4):
                nc.scalar.activation(
                    out=cT[:, t * 4 + og, :], in_=o_ps[og][:, :],
                    func=ACTF.Identity, bias=b2T[t][:, og:og + 1])

        # fw layer1: N-split into two col_grp chains (output halves)
        psf = {0: psB.tile([36, H // 2], F32, tag="ps1a", name="psfa"),
               1: psB.tile([36, H // 2], F32, tag="ps1b", name="psfb")}
        for half in range(2):
            nc.tensor.matmul(out=psf[half][32 * half:32 * half + BPC, :],
                             lhsT=onesb4,
                             rhs=fb1[:, half * (H // 2):(half + 1) * (H // 2)],
                             start=True, stop=False)
        for ch in range(8):
            for half in range(2):
                nc.tensor.matmul(
                    out=psf[half][32 * half:32 * half + BPC, :],
                    lhsT=cT[:, ch, :],
                    rhs=fw1[:, ch, half * (H // 2):(half + 1) * (H // 2)],
                    start=False, stop=(ch == 7))
        hf = state.tile([BPC, H], BF16, tag="hf")
        nc.vector.tensor_scalar_max(hf[:, 0:H // 2], psf[0][0:BPC, :], 0.0)
        nc.scalar.activation(out=hf[:, H // 2:],
                             in_=psf[1][32:32 + BPC, :], func=ACTF.Relu)

        # transpose hf [4, 512] -> hfT [128, 4ic, 4]
        hfT = state.tile([P, 4, BPC], BF16, tag="hfT")
        for ic2 in range(2):
            hfp = psT.tile([P, 2, BPC], BF16, tag="tr", name=f"hfp{ic2}")
            for j in range(2):
                ic = ic2 * 2 + j
                nc.tensor.transpose(out=hfp[:, j, :],
                                    in_=hf[:, ic * P:(ic + 1) * P],
                                    identity=identb_s[:BPC, :BPC])
            nc.vector.tensor_copy(hfT[:, ic2 * 2:ic2 * 2 + 2, :], hfp[:, :, :])

        # fw layer2 flipped -> resT [128, og, b]
        f_ps = [psO.tile([P, BPC], F32, tag="ops", name=f"fps{og}")
                for og in range(4)]
        for ic in range(4):
            for og in range(4):
                nc.tensor.matmul(
                    out=f_ps[og][:, :],
                    lhsT=fw2T[:, ic, og, :],
                    rhs=hfT[:, ic, :],
                    start=(ic == 0), stop=(ic == 3))
        resT = state.tile([P, 4, BPC], F32, tag="resT")
        for og in range(4):
            nc.scalar.activation(
                out=resT[:, og, :], in_=f_ps[og][:, :],
                func=ACTF.Identity, bias=fb2T[:, og:og + 1])
        nc.sync.dma_start(out=out[:, :],
                          in_=resT[:, :, :].rearrange("p a b -> p (a b)"))


_NC_CACHE = None


def _get_nc():
    global _NC_CACHE
    if _NC_CACHE is None:
        _NC_CACHE = build_nc()
    return _NC_CACHE


def _consts():
    cfm = np.zeros((P, CF_TOTAL), dtype=np.float32)
    cfm[:, CF_IDENT:CF_IDENT + P] = np.eye(P, dtype=np.float32)
    cfm[:, CF_PBASE] = np.arange(P, dtype=np.float32) * NP
    cfm[:, CF_BOFFS] = ((np.arange(P) // 64) * N).astype(np.float32)
    cfm[:, CF_MASK8:CF_MASK8 + 8] = (
        np.arange(P)[:, None] // 16 == np.arange(8)[None, :])
    cfm[:, CF_LREP:CF_LREP + P] = (
        np.arange(P)[:, None] % 16 == np.arange(P)[None, :] % 16)
    erep = np.zeros((NPAIR, NPAIR * P), dtype=np.float32)
    for q in range(NPAIR):
        erep[q, q * P:(q + 1) * P] = 1.0
    cfm[:NPAIR, CF_EREP:CF_EREP + NPAIR * P] = erep
    rr = np.arange(NPAIR * 8) % 8
    cfm[:NPAIR * 8, CF_MASKR] = (rr == 0)
    cfm[:NPAIR * 8, CF_MASKR + 1:CF_MASKR + 3] = (
        rr[:, None] == np.arange(1, 3)[None, :])
    cfm[:NPAIR * 8, CF_MASKR + 3:CF_MASKR + 8] = (
        rr[:, None] == np.arange(3, 8)[None, :])
    cfm[:NPAIR * 8, CF_SELQ:CF_SELQ + NPAIR] = (
        np.arange(NPAIR * 8)[:, None] // 8 == np.arange(NPAIR)[None, :])
    cbm = np.zeros((P, CB_TOTAL), dtype=np.float32)
    cbm[:, CB_IDENT:CB_IDENT + P] = np.eye(P)
    cbm[0, CB_ONES:CB_ONES + 8] = 1.0
    return cfm, cbm


def build_in_maps(points_xyz, point_features, joint_origin, drag_point,
                  jw1, jb1, jw2, jb2, dw1, db1, dw2, db2, fw1, fb1, fw2, fb2):
    from concurrent.futures import ThreadPoolExecutor

    cfm, cbm = _consts()
    # layer-2 biases, transposed feature-major, into cf
    cfm[:, CF_B2T0:CF_B2T0 + 4] = np.asarray(jb2, np.float32).reshape(4, P).T
    cfm[:, CF_B2T1:CF_B2T1 + 4] = np.asarray(db2, np.float32).reshape(4, P).T
    cfm[:, CF_FB2T:CF_FB2T + 4] = np.asarray(fb2, np.float32).reshape(4, P).T
    # layer-1 biases into cb row 0
    cbm[0, CB_B10:CB_B10 + H] = np.asarray(jb1, np.float32)
    cbm[0, CB_B11:CB_B11 + H] = np.asarray(db1, np.float32)
    cbm[0, CB_FB1:CB_FB1 + H] = np.asarray(fb1, np.float32)
    cbm = cbm.astype(BF)

    wm = np.empty((P, W_TOTAL), dtype=BF)

    def prep_w1(w):  # [1024, 512] -> [128, 8*512]
        w = np.asarray(w, np.float32)
        return np.ascontiguousarray(
            w.reshape(8, P, H).transpose(1, 0, 2).reshape(P, 8 * H)).astype(BF)

    def prep_w2T(w):  # [512, 512] -> [128, 4ic*4og*128]
        w = np.asarray(w, np.float32)
        return np.ascontiguousarray(
            w.reshape(4, P, 4, P).transpose(1, 0, 2, 3).reshape(P, 2048)
        ).astype(BF)

    wm[:, W_W10:W_W10 + 4096] = prep_w1(jw1)
    wm[:, W_W11:W_W11 + 4096] = prep_w1(dw1)
    wm[:, W_W2T0:W_W2T0 + 2048] = prep_w2T(jw2)
    wm[:, W_W2T1:W_W2T1 + 2048] = prep_w2T(dw2)
    wm[:, W_FW1:W_FW1 + 4096] = prep_w1(fw1)
    wm[:, W_FW2T:W_FW2T + 2048] = prep_w2T(fw2)

    pxyz = np.asarray(points_xyz, dtype=np.float32)
    pf = np.asarray(point_features)
    qj = np.asarray(joint_origin, dtype=np.float32)
    qd = np.asarray(drag_point, dtype=np.float32)

    def feats_half(args):
        c, hhalf = args
        buf = np.empty((2 * N, C), dtype=BF)
        for b2 in range(2):
            gb = c * BPC + hhalf * 2 + b2
            buf[b2 * N:(b2 + 1) * N] = pf[gb].T.astype(BF)
        return buf

    with ThreadPoolExecutor(max_workers=16) as ex:
        fhalves = list(ex.map(feats_half,
                              [(c, hh) for c in range(NCORES) for hh in range(2)]))

    in_maps = []
    for c in range(NCORES):
        sl = slice(c * BPC, (c + 1) * BPC)
        ptsc = np.ascontiguousarray(
            pxyz[sl].reshape(BPC, P, NP, 3).transpose(1, 0, 2, 3)
        ).reshape(P, BPC * NP * 3)
        qcat = np.concatenate([qj[sl], qd[sl]], axis=0).reshape(-1)
        qbc = np.broadcast_to(qcat[None, :], (P, NPAIR * 3))
        qpts0 = np.ascontiguousarray(
            np.concatenate([qbc, ptsc[:, :2 * NP * 3]], axis=1))
        qpts1 = np.ascontiguousarray(ptsc[:, 2 * NP * 3:])
        m = {"qpts0": qpts0, "qpts1": qpts1, "cf": cfm, "cb": cbm, "wts": wm,
             "feats0": fhalves[c * 2], "feats1": fhalves[c * 2 + 1]}
        in_maps.append(m)
    return in_maps


def kernel(**inputs):
    from concourse import bass_utils

    nc = _get_nc()
    in_maps = build_in_maps(**inputs)
    res = bass_utils.run_bass_kernel_spmd(nc, in_maps, core_ids=list(range(NCORES)))
    outs = []
    for r in res.results:
        # device layout [128, 4og, 4b] -> [4b, 512]
        o = r["out"].reshape(P, 4, BPC).transpose(2, 1, 0).reshape(BPC, OUT)
        outs.append(o)
    return np.concatenate(outs, axis=0)


# revision 26
# speedup vs baseline: 1.3120x; 1.3120x over previous
"""Trainium2 Bass kernel for LocalFeatureSamplerV10 (retrieval_knn), v4.

Full-input contract: kernel(**inputs) takes the complete unsharded numpy
inputs and returns the full [32, 512] output. Internally shards the batch
dim over 8 NeuronCores (4 batches/core), replicating the MLP weights.

v4 changes vs v3 (72.4us measured):
  * Distance phase: pairs processed as 4 couples (2 pairs batched per op,
    FD=768); subs and reduces split across vector/gpsimd; squares on
    scalar. Vector busy in stage A drops ~2.7x.
  * All constants + weights packed into 3 large DMAs on the sync queue
    (was 13 small ones, ~7.5us of gpsimd queue issue time freed).
  * MLP layer 1s col-tiled: t=0 at col_grp 0, t=1 at col_grp 1 run
    concurrently in the PE array (M=4 each); bias rank-1 matmuls open
    each accumulation group.
  * MLP layer 2s flipped: W2^T chunks [128h,128o] are the stationary
    operand, hT [128,4] the moving one -> output lands feature-major
    [128o, b], so no PE transposes of the layer-2 output, no activation
    copies; bias is fused into the PSUM->SBUF scalar copies (per-
    partition bias = b2 transposed). Final output written transposed
    [128, 4og, 4b] and unscrambled on host.
  * fw layer 1 N-split into two col_grp chains (halves of the 512 output
    cols), so consecutive matmuls never share a column group.
  * PSUM->SBUF copies (tv/cand/wB) moved to the scalar engine; pool
    reduces split gpsimd(chunk0)/vector(chunk1), dropping the v3
    scheduler-ordering hack.

Per-core algorithm (4 batches x 2 queries = 8 "pairs", pair = t*4 + b):
  1. s = -||p - q||^2 laid out [128 part, 128] per pair (point n = p*128+j).
  2. Top-32 per pair: per-partition top-8 (max8) -> PE-transpose candidates
     -> per-row top-32 (max + match_replace rounds) -> PE-transpose flatten
     -> global top-32 with the 8 pairs stacked on partitions (bit-exact).
  3. Indices via max_index against the original s rows + p*128, cross-
     partition min via PE transpose + reduce_min, clamped, + batch offset;
     FIND outputs land in permuted columns so the result is already in
     dma_gather's wrapped [16, n/16] table order.
  4. Two dma_gather(transpose=True) of 128 rows each from the bf16 feature
     stacks; reduce_max over K -> X [128ch, 8chhi, b, t] bf16.
  5. MLPs as bf16 PE matmuls (details above); fp32 output.
"""

import numpy as np
import ml_dtypes

import concourse.bass as bass
from concourse import bacc
import concourse.mybir as mybir
import concourse.tile as tile

B, N, C, K, OUT = 32, 16384, 1024, 32, 512
H = 512
NCORES = 8
BPC = B // NCORES          # batches per core
P = 128
NP = N // P                # 128 points per partition
NPAIR = 2 * BPC            # 8 (pair = t*BPC + b; 0-3 joint, 4-7 drag)
F32 = mybir.dt.float32
BF16 = mybir.dt.bfloat16
U32 = mybir.dt.uint32
I16 = mybir.dt.int16
NEG = -3.0e38

AX = mybir.AxisListType
OP = mybir.AluOpType
ACTF = mybir.ActivationFunctionType

BF = ml_dtypes.bfloat16

# ---- packed constant-f32 column offsets --------------------------------
CF_IDENT = 0
CF_PBASE = 128
CF_BOFFS = 129
CF_MASK8 = 130
CF_LREP = 138
CF_EREP = 266            # [8 rows, NPAIR*P=1024]
CF_MASKR = 1290          # [64 rows, 8]
CF_SELQ = 1298           # [64 rows, 8]
CF_B2T0 = 1306           # [128, 4]
CF_B2T1 = 1310
CF_FB2T = 1314
CF_TOTAL = 1318

# ---- packed constant-bf16 column offsets -------------------------------
CB_IDENT = 0
CB_ONES = 128            # 8 real ones (row 0)
CB_B10 = 136             # row 0 only, 512 cols
CB_B11 = 648
CB_FB1 = 1160
CB_TOTAL = 1672

# ---- packed weight-bf16 column offsets ---------------------------------
W_W10 = 0                # [128, 8, 512]
W_W11 = 4096
W_W2T0 = 8192            # [128, 4, 4, 128]
W_W2T1 = 10240
W_FW1 = 12288            # [128, 8, 512]
W_FW2T = 16384           # [128, 4, 4, 128]
W_TOTAL = 18432


def build_nc():
    nc = bacc.Bacc(trn_type="TRN2")

    qpts0 = nc.dram_tensor("qpts0", [P, NPAIR * 3 + 2 * NP * 3], F32,
                           kind="ExternalInput")
    qpts1 = nc.dram_tensor("qpts1", [P, 2 * NP * 3], F32, kind="ExternalInput")
    cf = nc.dram_tensor("cf", [P, CF_TOTAL], F32, kind="ExternalInput")
    cb = nc.dram_tensor("cb", [P, CB_TOTAL], BF16, kind="ExternalInput")
    wts = nc.dram_tensor("wts", [P, W_TOTAL], BF16, kind="ExternalInput")
    feats = [nc.dram_tensor(f"feats{h}", [2 * N, C], BF16, kind="ExternalInput")
             for h in range(2)]
    out = nc.dram_tensor("out", [P, 4 * BPC], F32, kind="ExternalOutput")

    with tile.TileContext(nc) as tc:
        _body(tc, nc, qpts0, qpts1, cf, cb, wts, feats, out)
    nc.compile()
    return nc


def _body(tc, nc, qpts0, qpts1, cf, cb, wts, feats, out):
    from contextlib import ExitStack
    with ExitStack() as ctx:
        cpool = ctx.enter_context(tc.tile_pool(name="const", bufs=1))
        wpool = ctx.enter_context(tc.tile_pool(name="weights", bufs=1))
        state = ctx.enter_context(tc.tile_pool(name="state", bufs=1))
        work = ctx.enter_context(tc.tile_pool(name="work", bufs=2))
        psA = ctx.enter_context(tc.tile_pool(name="psA", bufs=1, space="PSUM"))
        psB = ctx.enter_context(tc.tile_pool(name="psB", bufs=1, space="PSUM"))
        psT = ctx.enter_context(tc.tile_pool(name="psT", bufs=1, space="PSUM"))
        psO = ctx.enter_context(tc.tile_pool(name="psO", bufs=4, space="PSUM"))

        # ---- input DMAs, all on the sync queue ---------------------------
        # DMA1: queries + points batches 0,1; DMA2: points batches 2,3
        qp0 = state.tile([P, NPAIR * 3 + 2 * NP * 3], F32, tag="qp0")
        nc.sync.dma_start(out=qp0[:, :], in_=qpts0[:, :])
        qb_s = qp0[:, :NPAIR * 3].rearrange("p (i c) -> p i c", c=3)
        qp1 = state.tile([P, 2 * NP * 3], F32, tag="qp1")
        nc.sync.dma_start(out=qp1[:, :], in_=qpts1[:, :])

        def ptile(b):  # [P, NP, 3] view of batch b's points
            if b < 2:
                sl = qp0[:, NPAIR * 3 + b * NP * 3:NPAIR * 3 + (b + 1) * NP * 3]
            else:
                sl = qp1[:, (b - 2) * NP * 3:(b - 1) * NP * 3]
            return sl.rearrange("p (j c) -> p j c", c=3)

        # DMA3/4: packed constants
        cf_s = cpool.tile([P, CF_TOTAL], F32, tag="cf_s")
        nc.sync.dma_start(out=cf_s[:, :], in_=cf[:, :])
        cb_s = cpool.tile([P, CB_TOTAL], BF16, tag="cb_s")
        nc.sync.dma_start(out=cb_s[:, :], in_=cb[:, :])
        # DMA5: packed weights
        w_s = wpool.tile([P, W_TOTAL], BF16, tag="w_s")
        nc.sync.dma_start(out=w_s[:, :], in_=wts[:, :])

        ident = cf_s[:, CF_IDENT:CF_IDENT + P]
        pbase_s = cf_s[:, CF_PBASE:CF_PBASE + 1]
        boffs_s = cf_s[:, CF_BOFFS:CF_BOFFS + 1]
        mask8_s = cf_s[:, CF_MASK8:CF_MASK8 + 8]
        lrep_s = cf_s[:, CF_LREP:CF_LREP + P]
        erep_s = cf_s[:NPAIR, CF_EREP:CF_EREP + NPAIR * P]
        maskr_s = cf_s[:NPAIR * 8, CF_MASKR:CF_MASKR + 8]
        selq_s = cf_s[:NPAIR * 8, CF_SELQ:CF_SELQ + NPAIR]
        b2T = {0: cf_s[:, CF_B2T0:CF_B2T0 + 4], 1: cf_s[:, CF_B2T1:CF_B2T1 + 4]}
        fb2T = cf_s[:, CF_FB2T:CF_FB2T + 4]

        identb_s = cb_s[:, CB_IDENT:CB_IDENT + P]
        onesb4 = cb_s[:1, CB_ONES:CB_ONES + BPC]
        b1 = {0: cb_s[:1, CB_B10:CB_B10 + H], 1: cb_s[:1, CB_B11:CB_B11 + H]}
        fb1 = cb_s[:1, CB_FB1:CB_FB1 + H]

        w1 = {0: w_s[:, W_W10:W_W10 + 8 * H].rearrange("p (ch o) -> p ch o", ch=8),
              1: w_s[:, W_W11:W_W11 + 8 * H].rearrange("p (ch o) -> p ch o", ch=8)}
        w2T = {0: w_s[:, W_W2T0:W_W2T0 + 2048].rearrange(
                   "p (ic og o) -> p ic og o", ic=4, og=4),
               1: w_s[:, W_W2T1:W_W2T1 + 2048].rearrange(
                   "p (ic og o) -> p ic og o", ic=4, og=4)}
        fw1 = w_s[:, W_FW1:W_FW1 + 8 * H].rearrange("p (ch o) -> p ch o", ch=8)
        fw2T = w_s[:, W_FW2T:W_FW2T + 2048].rearrange(
            "p (ic og o) -> p ic og o", ic=4, og=4)

        # ---- PE warm-up scratch (shares psT's bank) ----------------------
        dps = psT.tile([NPAIR, P], F32, tag="tr", name="dps")

        def warm_f32(anchor_ap):
            kk = anchor_ap.shape[0]
            nc.tensor.matmul(out=dps[:anchor_ap.shape[1], :P], lhsT=anchor_ap,
                             rhs=ident[:kk, :], start=True, stop=True)

        def warm_bf16(anchor_ap):
            nc.tensor.matmul(out=dps[:anchor_ap.shape[1], :], lhsT=anchor_ap,
                             rhs=w1[0][:, 0, :P], start=True, stop=True)

        # ---- stage A: s = -d2 per couple (2 pairs), stage B: top-8 -------
        # couples: Cq = pairs (2q, 2q+1) i.e. (t, b01) with t=q//2? No:
        # pair = t*4+b. Couple layout below: (t, bpair) with bpair in {01, 23}.
        s_all = state.tile([P, NPAIR, NP], F32, tag="s_all")
        v8f = state.tile([P, NPAIR * 8], F32, tag="v8f")

        # couple index list: (t, b0) -> pairs (t*4+b0, t*4+b0+1)
        couples = [(0, 0), (1, 0), (1, 2), (0, 2)]  # emission order
        sub_eng = [nc.vector, nc.gpsimd, nc.vector, nc.gpsimd]
        red_eng = [nc.vector, nc.vector, nc.vector, nc.vector]

        for cidx, (t, b0) in enumerate(couples):
            i0 = t * BPC + b0
            diff = work.tile([P, 2, NP, 3], F32, tag="diff", name=f"diff{cidx}")
            # both batches' points, contiguous; queries broadcast per pair
            for k in range(2):
                sub_eng[cidx].tensor_sub(
                    out=diff[:, k, :, :], in0=ptile(b0 + k),
                    in1=qb_s[:, i0 + k:i0 + k + 1, :].to_broadcast([P, NP, 3]))
            sq = work.tile([P, 2, NP, 3], F32, tag="sq", name=f"sq{cidx}")
            nc.scalar.square(out=sq[:, :, :, :], in_=diff[:, :, :, :])
            red_eng[cidx].tensor_reduce(
                out=s_all[:, i0:i0 + 2, :], in_=sq[:, :, :, :],
                axis=AX.X, op=OP.add, negate=True)
            for k in range(2):
                i = i0 + k
                nc.vector.max(out=v8f[:, i * 8:(i + 1) * 8], in_=s_all[:, i, :])

        # ---- transpose candidates: [128, 64] -> [64, 128] ----------------
        tvp = psA.tile([NPAIR * 8, P], F32, tag="t64", name="tvp")
        nc.tensor.transpose(out=tvp[:, :], in_=v8f[:, :], identity=ident[:, :])
        tv = state.tile([NPAIR * 8, P], F32, tag="tv")
        nc.vector.tensor_copy(tv[:, :], tvp[:, :])

        # ---- stage C: per-row top-32 of candidates -----------------------
        cv = state.tile([NPAIR * 8, 32], F32, tag="cv")
        for r in range(4):
            sl = cv[:, r * 8:(r + 1) * 8]
            nc.vector.max(out=sl, in_=tv[:, :])
            if r < 3:
                nc.vector.match_replace(out=tv[:, :], in_to_replace=sl,
                                        in_values=tv[:, :], imm_value=NEG)

        # ---- flatten [64,32] -> [8,104] via masked matmul ----------------
        # rank-r of a partition can hold at most floor(32/(r+1)) winners, so
        # keep 32/16/16/8.. candidates per rank row (rounded to 8s) = 104.
        NCAND = 104
        cvmask = state.tile([NPAIR * 8, NCAND], F32, tag="cvmask")
        nc.vector.tensor_tensor(
            out=cvmask[:, 0:32], in0=cv[:, 0:32],
            in1=maskr_s[:, 0:1].to_broadcast([NPAIR * 8, 32]), op=OP.mult)
        nc.vector.tensor_tensor(
            out=cvmask[:, 32:64].rearrange("k (a c) -> k a c", a=2),
            in0=cv[:, 0:16].rearrange("k (a c) -> k a c", a=1).to_broadcast(
                [NPAIR * 8, 2, 16]),
            in1=maskr_s[:, 1:3].rearrange("k (r u) -> k r u", u=1).to_broadcast(
                [NPAIR * 8, 2, 16]),
            op=OP.mult)
        nc.vector.tensor_tensor(
            out=cvmask[:, 64:104].rearrange("k (a c) -> k a c", a=5),
            in0=cv[:, 0:8].rearrange("k (a c) -> k a c", a=1).to_broadcast(
                [NPAIR * 8, 5, 8]),
            in1=maskr_s[:, 3:8].rearrange("k (r u) -> k r u", u=1).to_broadcast(
                [NPAIR * 8, 5, 8]),
            op=OP.mult)
        candp = psA.tile([NPAIR, NCAND], F32, tag="t64", name="candp")
        nc.tensor.matmul(out=candp[:, :], lhsT=selq_s[:, :],
                         rhs=cvmask[:, :], start=True, stop=True)
        cand = state.tile([NPAIR, NCAND], F32, tag="cand")
        nc.vector.tensor_copy(cand[:, :], candp[:, :])

        # ---- stage D: global top-32 --------------------------------------
        wv = state.tile([NPAIR, 32], F32, tag="wv")
        for r in range(4):
            sl = wv[:, r * 8:(r + 1) * 8]
            nc.vector.max(out=sl, in_=cand[:, :])
            if r < 3:
                nc.vector.match_replace(out=cand[:, :], in_to_replace=sl,
                                        in_values=cand[:, :], imm_value=NEG)

        # ---- broadcast winners via per-pair selector matmuls -------------
        wBs = {}
        for q in [0, 1, 4, 5, 2, 3, 6, 7]:
            wbp = psA.tile([P, 32], F32, tag="t64", name=f"wbp{q}")
            nc.tensor.matmul(out=wbp[:, :],
                             lhsT=erep_s[:, q * P:(q + 1) * P],
                             rhs=wv[:, :], start=True, stop=True)
            wB = state.tile([P, 32], F32, tag=f"wB{q}", name=f"wB{q}")
            nc.scalar.copy(wB[:, :], wbp[:, :])
            wBs[q] = wB

        # ---- per 2-batch chunk: index recovery + gather + maxpool --------
        ju = state.tile([P, 2 * P], U32, tag="ju")
        jf = state.tile([P, 2 * P], F32, tag="jf")
        gfin = state.tile([P, 2], F32, tag="gfin")
        gcl = state.tile([P, 2], F32, tag="gcl")
        Xhs = [state.tile([P, 4, BPC, 2], BF16, tag=f"Xh{g}", name=f"Xh{g}")
               for g in range(2)]
        gmasks = []
        xgs = []
        idx_insts = []
        for hh in range(2):
            for t in range(2):
                for b2 in range(2):
                    i = t * BPC + 2 * hh + b2
                    for g in range(4):
                        col = hh * 128 + b2 * 64 + t * 32 + g * 8
                        nc.vector.max_index(out=ju[:, col:col + 8],
                                            in_max=wBs[i][:, g * 8:(g + 1) * 8],
                                            in_values=s_all[:, i, :])
            jfh = jf[:, hh * P:(hh + 1) * P]
            # cast u32->f32 and add p*128 in one DVE op
            nc.vector.scalar_tensor_tensor(
                out=jfh, in0=ju[:, hh * P:(hh + 1) * P], scalar=1.0,
                in1=pbase_s[:, :].to_broadcast([P, P]),
                op0=OP.mult, op1=OP.add)
            tp = psA.tile([P, P], F32, tag="t64", name=f"tp{hh}")
            nc.tensor.transpose(out=tp[:, :], in_=jfh, identity=ident[:, :])
            nc.vector.tensor_reduce(out=gfin[:, hh:hh + 1], in_=tp[:, :],
                                    axis=AX.X, op=OP.min)
            # clamp NOT_FOUND (huge) to N-1 and add per-slot batch offset
            nc.vector.scalar_tensor_tensor(
                out=gcl[:, hh:hh + 1], in0=gfin[:, hh:hh + 1],
                scalar=float(N - 1), in1=boffs_s[:, :],
                op0=OP.min, op1=OP.add)
            # wrapped+replicated idx table in one masked matmul
            gmask = state.tile([P, 8], F32, tag=f"gmask{hh}", name=f"gmask{hh}")
            gmasks.append(gmask)
            nc.vector.tensor_tensor(
                out=gmask[:, :], in0=gcl[:, hh:hh + 1].to_broadcast([P, 8]),
                in1=mask8_s[:, :], op=OP.mult)
            Tp = psA.tile([P, 8], F32, tag="t64", name=f"Tp{hh}")
            nc.tensor.matmul(out=Tp[:, :], lhsT=lrep_s[:, :], rhs=gmask[:, :],
                             start=True, stop=True)
            idx16 = state.tile([P, 8], I16, tag=f"idx16_{hh}", name=f"idx16_{hh}")
            idx_insts.append(nc.vector.tensor_copy(idx16[:, :], Tp[:, :]))
            xg = state.tile([P, 8, P], BF16, tag=f"xg{hh}", name=f"xg{hh}")
            nc.gpsimd.dma_gather(
                xg[:, :, :], feats[hh][:, :], idx16[:, :],
                num_idxs=P, num_idxs_reg=P, elem_size=C, transpose=True)
            xgs.append(xg)

        # maxpool over K (vector-only); g-major so L1 ch0-3 start early
        from bass_rust import InstructionNameOrderedSet
        for g in range(2):
            for hh in range(2):
                red = nc.vector.tensor_reduce(
                    out=Xhs[g][:, :, 2 * hh:2 * hh + 2, :],
                    in_=xgs[hh][:, g * 4:(g + 1) * 4, :].rearrange(
                        "p c8 (b2 t w) -> p c8 b2 t w", t=2, w=32),
                    axis=AX.X, op=OP.max)
                if hh == 0:
                    # ordering-only edge: keep chunk 0's reduces out of the
                    # vector stream until chunk 1's tail has issued
                    dep = InstructionNameOrderedSet()
                    dep.add(idx_insts[1].ins.name)
                    red.ins.add_nosync_dependencies_from(dep)

        # PE warm-up across the gather wait
        warm_f32(gmasks[1][:, :])
        for _ in range(14):
            warm_bf16(identb_s[:, :8])

        # ---- MLPs (bf16) -------------------------------------------------
        # t-layer1: col-tiled, t=0 -> rows 0:4 (col_grp 0), t=1 -> rows
        # 32:36 (col_grp 1), separate PSUM banks, concurrent in the array.
        ps1 = {0: psB.tile([36, H], F32, tag="ps1a", name="ps1a"),
               1: psB.tile([36, H], F32, tag="ps1b", name="ps1b")}
        for t in range(2):
            nc.tensor.matmul(out=ps1[t][32 * t:32 * t + BPC, :], lhsT=onesb4,
                             rhs=b1[t], start=True, stop=False)
        for ch in range(8):
            for t in range(2):
                nc.tensor.matmul(
                    out=ps1[t][32 * t:32 * t + BPC, :],
                    lhsT=Xhs[ch // 4][:, ch % 4, :, t],
                    rhs=w1[t][:, ch, :], start=False, stop=(ch == 7))
        hs = {}
        for t in range(2):
            ht = state.tile([BPC, H], BF16, tag=f"h{t}", name=f"h{t}")
            if t == 0:
                nc.vector.tensor_scalar_max(ht[:, :],
                                            ps1[t][0:BPC, :], 0.0)
            else:
                nc.scalar.activation(out=ht[:, :],
                                     in_=ps1[t][32:32 + BPC, :],
                                     func=ACTF.Relu)
            hs[t] = ht

        # transpose h pair [4, 512] -> hT [128, 4ic, (t,b)=8]
        hT = state.tile([P, 4, NPAIR], BF16, tag="hT")
        for ic2 in range(2):
            hTp = psT.tile([P, 2, NPAIR], BF16, tag="tr", name=f"hTp{ic2}")
            for j in range(2):
                ic = ic2 * 2 + j
                for t in range(2):
                    nc.tensor.transpose(
                        out=hTp[:, j, t * BPC:(t + 1) * BPC],
                        in_=hs[t][:, ic * P:(ic + 1) * P],
                        identity=identb_s[:BPC, :BPC])
            nc.vector.tensor_copy(hT[:, ic2 * 2:ic2 * 2 + 2, :], hTp[:, :, :])

        # t-layer2 flipped: stationary W2^T chunks, out feature-major.
        # 8 groups (t, og), 4 accumulating matmuls each, interleaved so
        # consecutive matmuls hit different PSUM tiles.
        # bias-fused copies target cT [128, (t,og)=8, b]
        cT = state.tile([P, 2 * 4, BPC], BF16, tag="cT")
        for t in range(2):
            o_ps = [psO.tile([P, BPC], F32, tag="ops", name=f"ops{t}{og}")
                    for og in range(4)]
            for ic in range(4):
                for og in range(4):
                    nc.tensor.matmul(
                        out=o_ps[og][:, :],
                        lhsT=w2T[t][:, ic, og, :],
                        rhs=hT[:, ic, t * BPC:(t + 1) * BPC],
                        start=(ic == 0), stop=(ic == 3))
            for og in range(4):
                nc.scalar.activation(
                    out=cT[:, t * 4 + og, :], in_=o_ps[og][:, :],
                    func=ACTF.Identity, bias=b2T[t][:, og:og + 1])

        # fw layer1: N-split into two col_grp chains (output halves)
        psf = {0: psB.tile([36, H // 2], F32, tag="ps1a", name="psfa"),
               1: psB.tile([36, H // 2], F32, tag="ps1b", name="psfb")}
        for half in range(2):
            nc.tensor.matmul(out=psf[half][32 * half:32 * half + BPC, :],
                             lhsT=onesb4,
                             rhs=fb1[:, half * (H // 2):(half + 1) * (H // 2)],
                             start=True, stop=False)
        for ch in range(8):
            for half in range(2):
                nc.tensor.matmul(
                    out=psf[half][32 * half:32 * half + BPC, :],
                    lhsT=cT[:, ch, :],
                    rhs=fw1[:, ch, half * (H // 2):(half + 1) * (H // 2)],
                    start=False, stop=(ch == 7))
        hf = state.tile([BPC, H], BF16, tag="hf")
        nc.vector.tensor_scalar_max(hf[:, 0:H // 2], psf[0][0:BPC, :], 0.0)
        nc.scalar.activation(out=hf[:, H // 2:],
                             in_=psf[1][32:32 + BPC, :], func=ACTF.Relu)

        # transpose hf [4, 512] -> hfT [128, 4ic, 4]
        hfT = state.tile([P, 4, BPC], BF16, tag="hfT")
        for ic2 in range(2):
            hfp = psT.tile([P, 2, BPC], BF16, tag="tr", name=f"hfp{ic2}")
            for j in range(2):
                ic = ic2 * 2 + j
                nc.tensor.transpose(out=hfp[:, j, :],
                                    in_=hf[:, ic * P:(ic + 1) * P],
                                    identity=identb_s[:BPC, :BPC])
            nc.vector.tensor_copy(hfT[:, ic2 * 2:ic2 * 2 + 2, :], hfp[:, :, :])

        # fw layer2 flipped -> resT [128, og, b]
        f_ps = [psO.tile([P, BPC], F32, tag="ops", name=f"fps{og}")
                for og in range(4)]
        for ic in range(4):
            for og in range(4):
                nc.tensor.matmul(
                    out=f_ps[og][:, :],
                    lhsT=fw2T[:, ic, og, :],
                    rhs=hfT[:, ic, :],
                    start=(ic == 0), stop=(ic == 3))
        resT = state.tile([P, 4, BPC], F32, tag="resT")
        for og in range(4):
            nc.scalar.activation(
                out=resT[:, og, :], in_=f_ps[og][:, :],
                func=ACTF.Identity, bias=fb2T[:, og:og + 1])
        nc.sync.dma_start(out=out[:, :],
                          in_=resT[:, :, :].rearrange("p a b -> p (a b)"))


_NC_CACHE = None


def _get_nc():
    global _NC_CACHE
    if _NC_CACHE is None:
        _NC_CACHE = build_nc()
    return _NC_CACHE


def _consts():
    cfm = np.zeros((P, CF_TOTAL), dtype=np.float32)
    cfm[:, CF_IDENT:CF_IDENT + P] = np.eye(P, dtype=np.float32)
    cfm[:, CF_PBASE] = np.arange(P, dtype=np.float32) * NP
    cfm[:, CF_BOFFS] = ((np.arange(P) // 64) * N).astype(np.float32)
    cfm[:, CF_MASK8:CF_MASK8 + 8] = (
        np.arange(P)[:, None] // 16 == np.arange(8)[None, :])
    cfm[:, CF_LREP:CF_LREP + P] = (
        np.arange(P)[:, None] % 16 == np.arange(P)[None, :] % 16)
    erep = np.zeros((NPAIR, NPAIR * P), dtype=np.float32)
    for q in range(NPAIR):
        erep[q, q * P:(q + 1) * P] = 1.0
    cfm[:NPAIR, CF_EREP:CF_EREP + NPAIR * P] = erep
    rr = np.arange(NPAIR * 8) % 8
    cfm[:NPAIR * 8, CF_MASKR] = (rr == 0)
    cfm[:NPAIR * 8, CF_MASKR + 1:CF_MASKR + 3] = (
        rr[:, None] == np.arange(1, 3)[None, :])
    cfm[:NPAIR * 8, CF_MASKR + 3:CF_MASKR + 8] = (
        rr[:, None] == np.arange(3, 8)[None, :])
    cfm[:NPAIR * 8, CF_SELQ:CF_SELQ + NPAIR] = (
        np.arange(NPAIR * 8)[:, None] // 8 == np.arange(NPAIR)[None, :])
    cbm = np.zeros((P, CB_TOTAL), dtype=np.float32)
    cbm[:, CB_IDENT:CB_IDENT + P] = np.eye(P)
    cbm[0, CB_ONES:CB_ONES + 8] = 1.0
    return cfm, cbm


def build_in_maps(points_xyz, point_features, joint_origin, drag_point,
                  jw1, jb1, jw2, jb2, dw1, db1, dw2, db2, fw1, fb1, fw2, fb2):
    from concurrent.futures import ThreadPoolExecutor

    cfm, cbm = _consts()
    # layer-2 biases, transposed feature-major, into cf
    cfm[:, CF_B2T0:CF_B2T0 + 4] = np.asarray(jb2, np.float32).reshape(4, P).T
    cfm[:, CF_B2T1:CF_B2T1 + 4] = np.asarray(db2, np.float32).reshape(4, P).T
    cfm[:, CF_FB2T:CF_FB2T + 4] = np.asarray(fb2, np.float32).reshape(4, P).T
    # layer-1 biases into cb row 0
    cbm[0, CB_B10:CB_B10 + H] = np.asarray(jb1, np.float32)
    cbm[0, CB_B11:CB_B11 + H] = np.asarray(db1, np.float32)
    cbm[0, CB_FB1:CB_FB1 + H] = np.asarray(fb1, np.float32)
    cbm = cbm.astype(BF)

    wm = np.empty((P, W_TOTAL), dtype=BF)

    def prep_w1(w):  # [1024, 512] -> [128, 8*512]
        w = np.asarray(w, np.float32)
        return np.ascontiguousarray(
            w.reshape(8, P, H).transpose(1, 0, 2).reshape(P, 8 * H)).astype(BF)

    def prep_w2T(w):  # [512, 512] -> [128, 4ic*4og*128]
        w = np.asarray(w, np.float32)
        return np.ascontiguousarray(
            w.reshape(4, P, 4, P).transpose(1, 0, 2, 3).reshape(P, 2048)
        ).astype(BF)

    wm[:, W_W10:W_W10 + 4096] = prep_w1(jw1)
    wm[:, W_W11:W_W11 + 4096] = prep_w1(dw1)
    wm[:, W_W2T0:W_W2T0 + 2048] = prep_w2T(jw2)
    wm[:, W_W2T1:W_W2T1 + 2048] = prep_w2T(dw2)
    wm[:, W_FW1:W_FW1 + 4096] = prep_w1(fw1)
    wm[:, W_FW2T:W_FW2T + 2048] = prep_w2T(fw2)

    pxyz = np.asarray(points_xyz, dtype=np.float32)
    pf = np.asarray(point_features)
    qj = np.asarray(joint_origin, dtype=np.float32)
    qd = np.asarray(drag_point, dtype=np.float32)

    def feats_half(args):
        c, hhalf = args
        buf = np.empty((2 * N, C), dtype=BF)
        for b2 in range(2):
            gb = c * BPC + hhalf * 2 + b2
            buf[b2 * N:(b2 + 1) * N] = pf[gb].T.astype(BF)
        return buf

    with ThreadPoolExecutor(max_workers=16) as ex:
        fhalves = list(ex.map(feats_half,
                              [(c, hh) for c in range(NCORES) for hh in range(2)]))

    in_maps = []
    for c in range(NCORES):
        sl = slice(c * BPC, (c + 1) * BPC)
        ptsc = np.ascontiguousarray(
            pxyz[sl].reshape(BPC, P, NP, 3).transpose(1, 0, 2, 3)
        ).reshape(P, BPC * NP * 3)
        qcat = np.concatenate([qj[sl], qd[sl]], axis=0).reshape(-1)
        qbc = np.broadcast_to(qcat[None, :], (P, NPAIR * 3))
        qpts0 = np.ascontiguousarray(
            np.concatenate([qbc, ptsc[:, :2 * NP * 3]], axis=1))
        qpts1 = np.ascontiguousarray(ptsc[:, 2 * NP * 3:])
        m = {"qpts0": qpts0, "qpts1": qpts1, "cf": cfm, "cb": cbm, "wts": wm,
             "feats0": fhalves[c * 2], "feats1": fhalves[c * 2 + 1]}
        in_maps.append(m)
    return in_maps


def kernel(**inputs):
    from concourse import bass_utils

    nc = _get_nc()
    in_maps = build_in_maps(**inputs)
    res = bass_utils.run_bass_kernel_spmd(nc, in_maps, core_ids=list(range(NCORES)))
    outs = []
    for r in res.results:
        # device layout [128, 4og, 4b] -> [4b, 512]
        o = r["out"].reshape(P, 4, BPC).transpose(2, 1, 0).reshape(BPC, OUT)
        outs.append(o)
    return np.concatenate(outs, axis=0)


# revision 27
# speedup vs baseline: 1.3283x; 1.0125x over previous
"""Trainium2 Bass kernel for LocalFeatureSamplerV10 (retrieval_knn), v5.

Full-input contract: kernel(**inputs) takes the complete unsharded numpy
inputs and returns the full [32, 512] output. Internally shards the batch
dim over 8 NeuronCores (4 batches/core), replicating the MLP weights.

v5 changes vs v3 (72.4us measured -> ~62-64us):
  * Distance phase: pairs processed as 4 couples (2 pairs batched per op,
    FD=768); subs split vector/gpsimd (C0,C3 / C1,C2); squares on scalar;
    reduces on vector.
  * All constants + weights packed into 5 large DMAs on the sync queue
    (gpsimd queue freed for distance work; fewer descriptor-gen stalls).
  * Rank-limited stage-D: rank-r of a partition can hold at most
    floor(32/(r+1)) global winners, so the flatten keeps 32/16/16/8x5
    = 104 candidates per pair instead of 256 (exact, cheaper rounds).
  * Index-recovery chain: u32->f32 cast fused into the +p*128
    scalar_tensor_tensor (one DVE op instead of two).
  * MLP layer 1s col-tiled: t=0 at col_grp 0, t=1 at col_grp 1 run
    concurrently in the PE array; bias rank-1 matmuls open each
    accumulation group; one PSUM bank per accumulation group (hardware
    clears has_written bank-wide on start).
  * MLP layer 2s flipped: W2^T chunks [128h,128o] are the stationary
    operand (FWL), hT [128,4] the moving one -> output lands feature-major
    [128o, b]: no output transposes, no activation copies; bias fused into
    the PSUM->SBUF scalar copies. Final output written transposed
    [128, 4og, 4b] and unscrambled on host.
  * fw layer 1 N-split into two col_grp chains (halves of the output).
  * PSUM->SBUF winner copies on the scalar engine; relu t0 on vector.
  * NOTE: gpsimd custom ops (tensor_tensor etc.) must not interleave with
    dma_gather - the Q7 library swap costs several us (measured 12us
    drain). Keep all gpsimd compute before the first gather.

Per-core algorithm (4 batches x 2 queries = 8 "pairs", pair = t*4 + b):
  1. s = -||p - q||^2 laid out [128 part, 128] per pair (point n = p*128+j).
  2. Top-32 per pair: per-partition top-8 (max8) -> PE-transpose candidates
     -> per-row top-k (max + match_replace rounds) -> masked-matmul flatten
     -> global top-32 with the 8 pairs stacked on partitions (bit-exact).
  3. Indices via max_index against the original s rows + p*128, cross-
     partition min via PE transpose + reduce_min, clamped, + batch offset;
     FIND outputs land in permuted columns matching dma_gather's wrapped
     [16, n/16] table order.
  4. Two dma_gather(transpose=True) of 128 rows each from the bf16 feature
     stacks; reduce_max over K -> X [128ch, 8chhi, b, t] bf16.
  5. MLPs as bf16 PE matmuls (see above); fp32 output.
"""

import numpy as np
import ml_dtypes

import concourse.bass as bass
from concourse import bacc
import concourse.mybir as mybir
import concourse.tile as tile

B, N, C, K, OUT = 32, 16384, 1024, 32, 512
H = 512
NCORES = 8
BPC = B // NCORES          # batches per core
P = 128
NP = N // P                # 128 points per partition
NPAIR = 2 * BPC            # 8 (pair = t*BPC + b; 0-3 joint, 4-7 drag)
F32 = mybir.dt.float32
BF16 = mybir.dt.bfloat16
U32 = mybir.dt.uint32
I16 = mybir.dt.int16
NEG = -3.0e38

AX = mybir.AxisListType
OP = mybir.AluOpType
ACTF = mybir.ActivationFunctionType

BF = ml_dtypes.bfloat16

# ---- packed constant-f32 column offsets --------------------------------
CF_IDENT = 0
CF_PBASE = 128
CF_BOFFS = 129
CF_MASK8 = 130
CF_LREP = 138
CF_EREP = 266            # [8 rows, NPAIR*P=1024]
CF_MASKR = 1290          # [64 rows, 8]
CF_SELQ = 1298           # [64 rows, 8]
CF_B2T0 = 1306           # [128, 4]
CF_B2T1 = 1310
CF_FB2T = 1314
CF_TOTAL = 1318

# ---- packed constant-bf16 column offsets -------------------------------
CB_IDENT = 0
CB_ONES = 128            # 8 real ones (row 0)
CB_B10 = 136             # row 0 only, 512 cols
CB_B11 = 648
CB_FB1 = 1160
CB_TOTAL = 1672

# ---- packed weight-bf16 column offsets ---------------------------------
W_W10 = 0                # [128, 8, 512]
W_W11 = 4096
W_W2T0 = 8192            # [128, 4, 4, 128]
W_W2T1 = 10240
W_FW1 = 12288            # [128, 8, 512]
W_FW2T = 16384           # [128, 4, 4, 128]
W_TOTAL = 18432


def build_nc():
    nc = bacc.Bacc(trn_type="TRN2")

    qpts0 = nc.dram_tensor("qpts0", [P, NPAIR * 3 + 2 * NP * 3], F32,
                           kind="ExternalInput")
    qpts1 = nc.dram_tensor("qpts1", [P, 2 * NP * 3], F32, kind="ExternalInput")
    cf = nc.dram_tensor("cf", [P, CF_TOTAL], F32, kind="ExternalInput")
    cb = nc.dram_tensor("cb", [P, CB_TOTAL], BF16, kind="ExternalInput")
    wts = nc.dram_tensor("wts", [P, W_TOTAL], BF16, kind="ExternalInput")
    feats = [nc.dram_tensor(f"feats{h}", [2 * N, C], BF16, kind="ExternalInput")
             for h in range(2)]
    out = nc.dram_tensor("out", [P, 4 * BPC], F32, kind="ExternalOutput")

    with tile.TileContext(nc) as tc:
        _body(tc, nc, qpts0, qpts1, cf, cb, wts, feats, out)
    nc.compile()
    return nc


def _body(tc, nc, qpts0, qpts1, cf, cb, wts, feats, out):
    from contextlib import ExitStack
    with ExitStack() as ctx:
        cpool = ctx.enter_context(tc.tile_pool(name="const", bufs=1))
        wpool = ctx.enter_context(tc.tile_pool(name="weights", bufs=1))
        state = ctx.enter_context(tc.tile_pool(name="state", bufs=1))
        work = ctx.enter_context(tc.tile_pool(name="work", bufs=2))
        psA = ctx.enter_context(tc.tile_pool(name="psA", bufs=1, space="PSUM"))
        psB = ctx.enter_context(tc.tile_pool(name="psB", bufs=1, space="PSUM"))
        psT = ctx.enter_context(tc.tile_pool(name="psT", bufs=1, space="PSUM"))
        psO = ctx.enter_context(tc.tile_pool(name="psO", bufs=4, space="PSUM"))

        # ---- input DMAs, all on the sync queue ---------------------------
        # DMA1: queries + points batches 0,1; DMA2: points batches 2,3
        qp0 = state.tile([P, NPAIR * 3 + 2 * NP * 3], F32, tag="qp0")
        nc.sync.dma_start(out=qp0[:, :], in_=qpts0[:, :])
        qb_s = qp0[:, :NPAIR * 3].rearrange("p (i c) -> p i c", c=3)
        qp1 = state.tile([P, 2 * NP * 3], F32, tag="qp1")
        nc.sync.dma_start(out=qp1[:, :], in_=qpts1[:, :])

        def ptile(b):  # [P, NP, 3] view of batch b's points
            if b < 2:
                sl = qp0[:, NPAIR * 3 + b * NP * 3:NPAIR * 3 + (b + 1) * NP * 3]
            else:
                sl = qp1[:, (b - 2) * NP * 3:(b - 1) * NP * 3]
            return sl.rearrange("p (j c) -> p j c", c=3)

        # DMA3/4: packed constants
        cf_s = cpool.tile([P, CF_TOTAL], F32, tag="cf_s")
        nc.sync.dma_start(out=cf_s[:, :], in_=cf[:, :])
        cb_s = cpool.tile([P, CB_TOTAL], BF16, tag="cb_s")
        nc.sync.dma_start(out=cb_s[:, :], in_=cb[:, :])
        # DMA5: packed weights
        w_s = wpool.tile([P, W_TOTAL], BF16, tag="w_s")
        nc.sync.dma_start(out=w_s[:, :], in_=wts[:, :])

        ident = cf_s[:, CF_IDENT:CF_IDENT + P]
        pbase_s = cf_s[:, CF_PBASE:CF_PBASE + 1]
        boffs_s = cf_s[:, CF_BOFFS:CF_BOFFS + 1]
        mask8_s = cf_s[:, CF_MASK8:CF_MASK8 + 8]
        lrep_s = cf_s[:, CF_LREP:CF_LREP + P]
        erep_s = cf_s[:NPAIR, CF_EREP:CF_EREP + NPAIR * P]
        maskr_s = cf_s[:NPAIR * 8, CF_MASKR:CF_MASKR + 8]
        selq_s = cf_s[:NPAIR * 8, CF_SELQ:CF_SELQ + NPAIR]
        b2T = {0: cf_s[:, CF_B2T0:CF_B2T0 + 4], 1: cf_s[:, CF_B2T1:CF_B2T1 + 4]}
        fb2T = cf_s[:, CF_FB2T:CF_FB2T + 4]

        identb_s = cb_s[:, CB_IDENT:CB_IDENT + P]
        onesb4 = cb_s[:1, CB_ONES:CB_ONES + BPC]
        b1 = {0: cb_s[:1, CB_B10:CB_B10 + H], 1: cb_s[:1, CB_B11:CB_B11 + H]}
        fb1 = cb_s[:1, CB_FB1:CB_FB1 + H]

        w1 = {0: w_s[:, W_W10:W_W10 + 8 * H].rearrange("p (ch o) -> p ch o", ch=8),
              1: w_s[:, W_W11:W_W11 + 8 * H].rearrange("p (ch o) -> p ch o", ch=8)}
        w2T = {0: w_s[:, W_W2T0:W_W2T0 + 2048].rearrange(
                   "p (ic og o) -> p ic og o", ic=4, og=4),
               1: w_s[:, W_W2T1:W_W2T1 + 2048].rearrange(
                   "p (ic og o) -> p ic og o", ic=4, og=4)}
        fw1 = w_s[:, W_FW1:W_FW1 + 8 * H].rearrange("p (ch o) -> p ch o", ch=8)
        fw2T = w_s[:, W_FW2T:W_FW2T + 2048].rearrange(
            "p (ic og o) -> p ic og o", ic=4, og=4)

        # ---- PE warm-up scratch (shares psT's bank) ----------------------
        dps = psT.tile([NPAIR, P], F32, tag="tr", name="dps")

        def warm_f32(anchor_ap):
            kk = anchor_ap.shape[0]
            nc.tensor.matmul(out=dps[:anchor_ap.shape[1], :P], lhsT=anchor_ap,
                             rhs=ident[:kk, :], start=True, stop=True)

        def warm_bf16(anchor_ap):
            nc.tensor.matmul(out=dps[:anchor_ap.shape[1], :], lhsT=anchor_ap,
                             rhs=w1[0][:, 0, :P], start=True, stop=True)

        # ---- stage A: s = -d2 per couple (2 pairs), stage B: top-8 -------
        # couples: Cq = pairs (2q, 2q+1) i.e. (t, b01) with t=q//2? No:
        # pair = t*4+b. Couple layout below: (t, bpair) with bpair in {01, 23}.
        s_all = state.tile([P, NPAIR, NP], F32, tag="s_all")
        v8f = state.tile([P, NPAIR * 8], F32, tag="v8f")

        # couple index list: (t, b0) -> pairs (t*4+b0, t*4+b0+1)
        couples = [(0, 0), (1, 0), (1, 2), (0, 2)]  # emission order
        sub_eng = [nc.vector, nc.gpsimd, nc.vector, nc.gpsimd]
        red_eng = [nc.vector, nc.vector, nc.vector, nc.vector]

        for cidx, (t, b0) in enumerate(couples):
            i0 = t * BPC + b0
            diff = work.tile([P, 2, NP, 3], F32, tag="diff", name=f"diff{cidx}")
            # both batches' points, contiguous; queries broadcast per pair
            for k in range(2):
                sub_eng[cidx].tensor_sub(
                    out=diff[:, k, :, :], in0=ptile(b0 + k),
                    in1=qb_s[:, i0 + k:i0 + k + 1, :].to_broadcast([P, NP, 3]))
            sq = work.tile([P, 2, NP, 3], F32, tag="sq", name=f"sq{cidx}")
            nc.scalar.square(out=sq[:, :, :, :], in_=diff[:, :, :, :])
            red_eng[cidx].tensor_reduce(
                out=s_all[:, i0:i0 + 2, :], in_=sq[:, :, :, :],
                axis=AX.X, op=OP.add, negate=True)
            for k in range(2):
                i = i0 + k
                nc.vector.max(out=v8f[:, i * 8:(i + 1) * 8], in_=s_all[:, i, :])

        # ---- transpose candidates: [128, 64] -> [64, 128] ----------------
        tvp = psA.tile([NPAIR * 8, P], F32, tag="t64", name="tvp")
        nc.tensor.transpose(out=tvp[:, :], in_=v8f[:, :], identity=ident[:, :])
        tv = state.tile([NPAIR * 8, P], F32, tag="tv")
        nc.vector.tensor_copy(tv[:, :], tvp[:, :])

        # ---- stage C: per-row top-32 of candidates -----------------------
        cv = state.tile([NPAIR * 8, 32], F32, tag="cv")
        for r in range(4):
            sl = cv[:, r * 8:(r + 1) * 8]
            nc.vector.max(out=sl, in_=tv[:, :])
            if r < 3:
                nc.vector.match_replace(out=tv[:, :], in_to_replace=sl,
                                        in_values=tv[:, :], imm_value=NEG)

        # ---- flatten [64,32] -> [8,104] via masked matmul ----------------
        # rank-r of a partition can hold at most floor(32/(r+1)) winners, so
        # keep 32/16/16/8.. candidates per rank row (rounded to 8s) = 104.
        NCAND = 104
        cvmask = state.tile([NPAIR * 8, NCAND], F32, tag="cvmask")
        nc.vector.tensor_tensor(
            out=cvmask[:, 0:32], in0=cv[:, 0:32],
            in1=maskr_s[:, 0:1].to_broadcast([NPAIR * 8, 32]), op=OP.mult)
        nc.vector.tensor_tensor(
            out=cvmask[:, 32:64].rearrange("k (a c) -> k a c", a=2),
            in0=cv[:, 0:16].rearrange("k (a c) -> k a c", a=1).to_broadcast(
                [NPAIR * 8, 2, 16]),
            in1=maskr_s[:, 1:3].rearrange("k (r u) -> k r u", u=1).to_broadcast(
                [NPAIR * 8, 2, 16]),
            op=OP.mult)
        nc.vector.tensor_tensor(
            out=cvmask[:, 64:104].rearrange("k (a c) -> k a c", a=5),
            in0=cv[:, 0:8].rearrange("k (a c) -> k a c", a=1).to_broadcast(
                [NPAIR * 8, 5, 8]),
            in1=maskr_s[:, 3:8].rearrange("k (r u) -> k r u", u=1).to_broadcast(
                [NPAIR * 8, 5, 8]),
            op=OP.mult)
        candp = psA.tile([NPAIR, NCAND], F32, tag="t64", name="candp")
        nc.tensor.matmul(out=candp[:, :], lhsT=selq_s[:, :],
                         rhs=cvmask[:, :], start=True, stop=True)
        cand = state.tile([NPAIR, NCAND], F32, tag="cand")
        nc.vector.tensor_copy(cand[:, :], candp[:, :])

        # ---- stage D: global top-32 --------------------------------------
        wv = state.tile([NPAIR, 32], F32, tag="wv")
        for r in range(4):
            sl = wv[:, r * 8:(r + 1) * 8]
            nc.vector.max(out=sl, in_=cand[:, :])
            if r < 3:
                nc.vector.match_replace(out=cand[:, :], in_to_replace=sl,
                                        in_values=cand[:, :], imm_value=NEG)

        # ---- broadcast winners via per-pair selector matmuls -------------
        wBs = {}
        for q in [0, 1, 4, 5, 2, 3, 6, 7]:
            wbp = psA.tile([P, 32], F32, tag="t64", name=f"wbp{q}")
            nc.tensor.matmul(out=wbp[:, :],
                             lhsT=erep_s[:, q * P:(q + 1) * P],
                             rhs=wv[:, :], start=True, stop=True)
            wB = state.tile([P, 32], F32, tag=f"wB{q}", name=f"wB{q}")
            nc.scalar.copy(wB[:, :], wbp[:, :])
            wBs[q] = wB

        # ---- per 2-batch chunk: index recovery + gather + maxpool --------
        ju = state.tile([P, 2 * P], U32, tag="ju")
        jf = state.tile([P, 2 * P], F32, tag="jf")
        gfin = state.tile([P, 2], F32, tag="gfin")
        gcl = state.tile([P, 2], F32, tag="gcl")
        Xhs = [state.tile([P, 4, BPC, 2], BF16, tag=f"Xh{g}", name=f"Xh{g}")
               for g in range(2)]
        gmasks = []
        xgs = []
        idx_insts = []
        for hh in range(2):
            for t in range(2):
                for b2 in range(2):
                    i = t * BPC + 2 * hh + b2
                    for g in range(4):
                        col = hh * 128 + b2 * 64 + t * 32 + g * 8
                        nc.vector.max_index(out=ju[:, col:col + 8],
                                            in_max=wBs[i][:, g * 8:(g + 1) * 8],
                                            in_values=s_all[:, i, :])
            jfh = jf[:, hh * P:(hh + 1) * P]
            # cast u32->f32 and add p*128 in one DVE op
            nc.vector.scalar_tensor_tensor(
                out=jfh, in0=ju[:, hh * P:(hh + 1) * P], scalar=1.0,
                in1=pbase_s[:, :].to_broadcast([P, P]),
                op0=OP.mult, op1=OP.add)
            tp = psA.tile([P, P], F32, tag="t64", name=f"tp{hh}")
            nc.tensor.transpose(out=tp[:, :], in_=jfh, identity=ident[:, :])
            nc.vector.tensor_reduce(out=gfin[:, hh:hh + 1], in_=tp[:, :],
                                    axis=AX.X, op=OP.min)
            # clamp NOT_FOUND (huge) to N-1 and add per-slot batch offset
            nc.vector.scalar_tensor_tensor(
                out=gcl[:, hh:hh + 1], in0=gfin[:, hh:hh + 1],
                scalar=float(N - 1), in1=boffs_s[:, :],
                op0=OP.min, op1=OP.add)
            # wrapped+replicated idx table in one masked matmul
            gmask = state.tile([P, 8], F32, tag=f"gmask{hh}", name=f"gmask{hh}")
            gmasks.append(gmask)
            nc.vector.tensor_tensor(
                out=gmask[:, :], in0=gcl[:, hh:hh + 1].to_broadcast([P, 8]),
                in1=mask8_s[:, :], op=OP.mult)
            Tp = psA.tile([P, 8], F32, tag="t64", name=f"Tp{hh}")
            nc.tensor.matmul(out=Tp[:, :], lhsT=lrep_s[:, :], rhs=gmask[:, :],
                             start=True, stop=True)
            idx16 = state.tile([P, 8], I16, tag=f"idx16_{hh}", name=f"idx16_{hh}")
            idx_insts.append(nc.vector.tensor_copy(idx16[:, :], Tp[:, :]))
            xg = state.tile([P, 8, P], BF16, tag=f"xg{hh}", name=f"xg{hh}")
            nc.gpsimd.dma_gather(
                xg[:, :, :], feats[hh][:, :], idx16[:, :],
                num_idxs=P, num_idxs_reg=P, elem_size=C, transpose=True)
            xgs.append(xg)

        # maxpool over K (vector-only); g-major so L1 ch0-3 start early
        from bass_rust import InstructionNameOrderedSet
        for g in range(2):
            for hh in range(2):
                red = nc.vector.tensor_reduce(
                    out=Xhs[g][:, :, 2 * hh:2 * hh + 2, :],
                    in_=xgs[hh][:, g * 4:(g + 1) * 4, :].rearrange(
                        "p c8 (b2 t w) -> p c8 b2 t w", t=2, w=32),
                    axis=AX.X, op=OP.max)
                if hh == 0:
                    # ordering-only edge: keep chunk 0's reduces out of the
                    # vector stream until chunk 1's tail has issued
                    dep = InstructionNameOrderedSet()
                    dep.add(idx_insts[1].ins.name)
                    red.ins.add_nosync_dependencies_from(dep)

        # PE warm-up across the gather wait
        warm_f32(gmasks[1][:, :])
        for _ in range(14):
            warm_bf16(identb_s[:, :8])

        # ---- MLPs (bf16) -------------------------------------------------
        # t-layer1: col-tiled, t=0 -> rows 0:4 (col_grp 0), t=1 -> rows
        # 32:36 (col_grp 1), separate PSUM banks, concurrent in the array.
        ps1 = {0: psB.tile([36, H], F32, tag="ps1a", name="ps1a"),
               1: psB.tile([36, H], F32, tag="ps1b", name="ps1b")}
        for t in range(2):
            nc.tensor.matmul(out=ps1[t][32 * t:32 * t + BPC, :], lhsT=onesb4,
                             rhs=b1[t], start=True, stop=False)
        for ch in range(8):
            for t in range(2):
                nc.tensor.matmul(
                    out=ps1[t][32 * t:32 * t + BPC, :],
                    lhsT=Xhs[ch // 4][:, ch % 4, :, t],
                    rhs=w1[t][:, ch, :], start=False, stop=(ch == 7))
        hs = {}
        for t in range(2):
            ht = state.tile([BPC, H], BF16, tag=f"h{t}", name=f"h{t}")
            if t == 0:
                nc.vector.tensor_scalar_max(ht[:, :],
                                            ps1[t][0:BPC, :], 0.0)
            else:
                nc.scalar.activation(out=ht[:, :],
                                     in_=ps1[t][32:32 + BPC, :],
                                     func=ACTF.Relu)
            hs[t] = ht

        # transpose h pair [4, 512] -> hT [128, 4ic, (t,b)=8]
        hT = state.tile([P, 4, NPAIR], BF16, tag="hT")
        for ic2 in range(2):
            hTp = psT.tile([P, 2, NPAIR], BF16, tag="tr", name=f"hTp{ic2}")
            for j in range(2):
                ic = ic2 * 2 + j
                for t in range(2):
                    nc.tensor.transpose(
                        out=hTp[:, j, t * BPC:(t + 1) * BPC],
                        in_=hs[t][:, ic * P:(ic + 1) * P],
                        identity=identb_s[:BPC, :BPC])
            nc.vector.tensor_copy(hT[:, ic2 * 2:ic2 * 2 + 2, :], hTp[:, :, :])

        # t-layer2 flipped: stationary W2^T chunks, out feature-major.
        # 8 groups (t, og), 4 accumulating matmuls each, interleaved so
        # consecutive matmuls hit different PSUM tiles.
        # bias-fused copies target cT [128, (t,og)=8, b]
        cT = state.tile([P, 2 * 4, BPC], BF16, tag="cT")
        for t in range(2):
            o_ps = [psO.tile([P, BPC], F32, tag="ops", name=f"ops{t}{og}")
                    for og in range(4)]
            for ic in range(4):
                for og in range(4):
                    nc.tensor.matmul(
                        out=o_ps[og][:, :],
                        lhsT=w2T[t][:, ic, og, :],
                        rhs=hT[:, ic, t * BPC:(t + 1) * BPC],
                        start=(ic == 0), stop=(ic == 3))
            for og in range(4):
                nc.scalar.activation(
                    out=cT[:, t * 4 + og, :], in_=o_ps[og][:, :],
                    func=ACTF.Identity, bias=b2T[t][:, og:og + 1])

        # fw layer1: N-split into two col_grp chains (output halves)
        psf = {0: psB.tile([36, H // 2], F32, tag="ps1a", name="psfa"),
               1: psB.tile([36, H // 2], F32, tag="ps1b", name="psfb")}
        for half in range(2):
            nc.tensor.matmul(out=psf[half][32 * half:32 * half + BPC, :],
                             lhsT=onesb4,
                             rhs=fb1[:, half * (H // 2):(half + 1) * (H // 2)],
                             start=True, stop=False)
        for ch in range(8):
            for half in range(2):
                nc.tensor.matmul(
                    out=psf[half][32 * half:32 * half + BPC, :],
                    lhsT=cT[:, ch, :],
                    rhs=fw1[:, ch, half * (H // 2):(half + 1) * (H // 2)],
                    start=False, stop=(ch == 7))
        hf = state.tile([BPC, H], BF16, tag="hf")
        nc.vector.tensor_scalar_max(hf[:, 0:H // 2], psf[0][0:BPC, :], 0.0)
        nc.scalar.activation(out=hf[:, H // 2:],
                             in_=psf[1][32:32 + BPC, :], func=ACTF.Relu)

        # transpose hf [4, 512] -> hfT [128, 4ic, 4]
        hfT = state.tile([P, 4, BPC], BF16, tag="hfT")
        for ic2 in range(2):
            hfp = psT.tile([P, 2, BPC], BF16, tag="tr", name=f"hfp{ic2}")
            for j in range(2):
                ic = ic2 * 2 + j
                nc.tensor.transpose(out=hfp[:, j, :],
                                    in_=hf[:, ic * P:(ic + 1) * P],
                                    identity=identb_s[:BPC, :BPC])
            nc.vector.tensor_copy(hfT[:, ic2 * 2:ic2 * 2 + 2, :], hfp[:, :, :])

        # fw layer2 flipped -> resT [128, og, b]
        f_ps = [psO.tile([P, BPC], F32, tag="ops", name=f"fps{og}")
                for og in range(4)]
        for ic in range(4):
            for og in range(4):
                nc.tensor.matmul(
                    out=f_ps[og][:, :],
                    lhsT=fw2T[:, ic, og, :],
                    rhs=hfT[:, ic, :],
                    start=(ic == 0), stop=(ic == 3))
        resT = state.tile([P, 4, BPC], F32, tag="resT")
        for og in range(4):
            nc.scalar.activation(
                out=resT[:, og, :], in_=f_ps[og][:, :],
                func=ACTF.Identity, bias=fb2T[:, og:og + 1])
        nc.sync.dma_start(out=out[:, :],
                          in_=resT[:, :, :].rearrange("p a b -> p (a b)"))


_NC_CACHE = None


def _get_nc():
    global _NC_CACHE
    if _NC_CACHE is None:
        _NC_CACHE = build_nc()
    return _NC_CACHE


def _consts():
    cfm = np.zeros((P, CF_TOTAL), dtype=np.float32)
    cfm[:, CF_IDENT:CF_IDENT + P] = np.eye(P, dtype=np.float32)
    cfm[:, CF_PBASE] = np.arange(P, dtype=np.float32) * NP
    cfm[:, CF_BOFFS] = ((np.arange(P) // 64) * N).astype(np.float32)
    cfm[:, CF_MASK8:CF_MASK8 + 8] = (
        np.arange(P)[:, None] // 16 == np.arange(8)[None, :])
    cfm[:, CF_LREP:CF_LREP + P] = (
        np.arange(P)[:, None] % 16 == np.arange(P)[None, :] % 16)
    erep = np.zeros((NPAIR, NPAIR * P), dtype=np.float32)
    for q in range(NPAIR):
        erep[q, q * P:(q + 1) * P] = 1.0
    cfm[:NPAIR, CF_EREP:CF_EREP + NPAIR * P] = erep
    rr = np.arange(NPAIR * 8) % 8
    cfm[:NPAIR * 8, CF_MASKR] = (rr == 0)
    cfm[:NPAIR * 8, CF_MASKR + 1:CF_MASKR + 3] = (
        rr[:, None] == np.arange(1, 3)[None, :])
    cfm[:NPAIR * 8, CF_MASKR + 3:CF_MASKR + 8] = (
        rr[:, None] == np.arange(3, 8)[None, :])
    cfm[:NPAIR * 8, CF_SELQ:CF_SELQ + NPAIR] = (
        np.arange(NPAIR * 8)[:, None] // 8 == np.arange(NPAIR)[None, :])
    cbm = np.zeros((P, CB_TOTAL), dtype=np.float32)
    cbm[:, CB_IDENT:CB_IDENT + P] = np.eye(P)
    cbm[0, CB_ONES:CB_ONES + 8] = 1.0
    return cfm, cbm


def build_in_maps(points_xyz, point_features, joint_origin, drag_point,
                  jw1, jb1, jw2, jb2, dw1, db1, dw2, db2, fw1, fb1, fw2, fb2):
    from concurrent.futures import ThreadPoolExecutor

    cfm, cbm = _consts()
    # layer-2 biases, transposed feature-major, into cf
    cfm[:, CF_B2T0:CF_B2T0 + 4] = np.asarray(jb2, np.float32).reshape(4, P).T
    cfm[:, CF_B2T1:CF_B2T1 + 4] = np.asarray(db2, np.float32).reshape(4, P).T
    cfm[:, CF_FB2T:CF_FB2T + 4] = np.asarray(fb2, np.float32).reshape(4, P).T
    # layer-1 biases into cb row 0
    cbm[0, CB_B10:CB_B10 + H] = np.asarray(jb1, np.float32)
    cbm[0, CB_B11:CB_B11 + H] = np.asarray(db1, np.float32)
    cbm[0, CB_FB1:CB_FB1 + H] = np.asarray(fb1, np.float32)
    cbm = cbm.astype(BF)

    wm = np.empty((P, W_TOTAL), dtype=BF)

    def prep_w1(w):  # [1024, 512] -> [128, 8*512]
        w = np.asarray(w, np.float32)
        return np.ascontiguousarray(
            w.reshape(8, P, H).transpose(1, 0, 2).reshape(P, 8 * H)).astype(BF)

    def prep_w2T(w):  # [512, 512] -> [128, 4ic*4og*128]
        w = np.asarray(w, np.float32)
        return np.ascontiguousarray(
            w.reshape(4, P, 4, P).transpose(1, 0, 2, 3).reshape(P, 2048)
        ).astype(BF)

    wm[:, W_W10:W_W10 + 4096] = prep_w1(jw1)
    wm[:, W_W11:W_W11 + 4096] = prep_w1(dw1)
    wm[:, W_W2T0:W_W2T0 + 2048] = prep_w2T(jw2)
    wm[:, W_W2T1:W_W2T1 + 2048] = prep_w2T(dw2)
    wm[:, W_FW1:W_FW1 + 4096] = prep_w1(fw1)
    wm[:, W_FW2T:W_FW2T + 2048] = prep_w2T(fw2)

    pxyz = np.asarray(points_xyz, dtype=np.float32)
    pf = np.asarray(point_features)
    qj = np.asarray(joint_origin, dtype=np.float32)
    qd = np.asarray(drag_point, dtype=np.float32)

    def feats_half(args):
        c, hhalf = args
        buf = np.empty((2 * N, C), dtype=BF)
        for b2 in range(2):
            gb = c * BPC + hhalf * 2 + b2
            buf[b2 * N:(b2 + 1) * N] = pf[gb].T.astype(BF)
        return buf

    with ThreadPoolExecutor(max_workers=16) as ex:
        fhalves = list(ex.map(feats_half,
                              [(c, hh) for c in range(NCORES) for hh in range(2)]))

    in_maps = []
    for c in range(NCORES):
        sl = slice(c * BPC, (c + 1) * BPC)
        ptsc = np.ascontiguousarray(
            pxyz[sl].reshape(BPC, P, NP, 3).transpose(1, 0, 2, 3)
        ).reshape(P, BPC * NP * 3)
        qcat = np.concatenate([qj[sl], qd[sl]], axis=0).reshape(-1)
        qbc = np.broadcast_to(qcat[None, :], (P, NPAIR * 3))
        qpts0 = np.ascontiguousarray(
            np.concatenate([qbc, ptsc[:, :2 * NP * 3]], axis=1))
        qpts1 = np.ascontiguousarray(ptsc[:, 2 * NP * 3:])
        m = {"qpts0": qpts0, "qpts1": qpts1, "cf": cfm, "cb": cbm, "wts": wm,
             "feats0": fhalves[c * 2], "feats1": fhalves[c * 2 + 1]}
        in_maps.append(m)
    return in_maps


def kernel(**inputs):
    from concourse import bass_utils

    nc = _get_nc()
    in_maps = build_in_maps(**inputs)
    res = bass_utils.run_bass_kernel_spmd(nc, in_maps, core_ids=list(range(NCORES)))
    outs = []
    for r in res.results:
        # device layout [128, 4og, 4b] -> [4b, 512]
        o = r["out"].reshape(P, 4, BPC).transpose(2, 1, 0).reshape(BPC, OUT)
        outs.append(o)
    return np.concatenate(outs, axis=0)
